# revision 16
# baseline (speedup 1.0000x reference)
"""Trainium2 Bass kernel for the Bayesian logistic-regression activation matrix.

Computes, for x [N, D], w_mu [D], w_log_var [D], z [NS]:
    mean  = x @ w_mu                       [N]
    var   = (x*x) @ exp(w_log_var)         [N]
    out[i, j] = sqrt(var_i) * z_j + mean_i [N, NS]

Data-parallel over 8 NeuronCores: rows of x sharded, everything else
replicated. The problem is HBM-bound; all device DMAs are shaped so every
descriptor element is a >=1KB contiguous run (no AP-transpose DMAs, which
degenerate to 4B packets):

  - x is cast to bf16 AND pre-transposed on the host into per-tile slabs
    [128 p, 4 c, r n] (d = 128c+p on partitions). Each tile loads with ONE
    dma_start, 2KB-4KB contiguous per partition. Halves HBM read traffic
    vs f32 and puts the D-reduction on the partition axis.
  - The first 512 rows are 4 mini-tiles (r=128) so the software pipeline
    fills in ~1us instead of waiting on a full 512KB load + square.
  - x^2 in fp8e4 (values < 36 << 240), split DVE (tensor_tensor) / ACT
    (Square activation) to balance engine load.
  - PE reduces over d: mean = sum_c w_c^T @ x_c (bf16, 4 chunk matmuls),
    var = e^T @ sq via TWO DoubleRow fp8 matmuls (virtual K=256: chunk
    pairs ride the Ko=2 AP dim, halving the moving-stream cycles).
  - ACT: std = Sqrt(psum_var) -> bf16; DVE: mean row -> bf16.
  - PE output, quad-interleaved for full tiles: block q computes rows
    n0 + 4p + q via K=1 outer products with stride-4 mean/std slices as
    stationary operands and [1..1] / z as the moving operand.
  - DVE evicts psum -> SBUF bf16; ONE fused store DMA per tile (1024B
    contiguous per partition on full tiles). Output is bf16 (halves store
    traffic); host upcasts to f32 (quantization ~0.4% of absmax, well
    under the 2e-2 gate).
  - Pipeline is software-skewed (load k | square k-1 | reduce k-2 |
    expand/store k-3) so no engine queue head waits on same-iteration work
    and PE gaps stay far below the ~3.4us HAM re-throttle window.
  - exp(w_log_var) and all tiny-vector prep happen on host ([512]/[128]).

The tail tile overlaps the previous one (rows 11988..12500 vs 11776..12288):
overlapping rows are recomputed from identical inputs with identical
instruction sequences, so both stores write identical bytes.
"""

import numpy as np

N = 100000
D = 512
NS = 128
NCORES = 8
NSHARD = N // NCORES  # 12500 rows per core
P = 128  # SBUF partitions
C = D // P  # 4 chunks of the feature dim
RF = 512  # full-tile rows (psum bank = 512 fp32)
RS = 128  # fill-tile rows
NSMALL = 0
NBIG = 25  # 24 mid + 1 overlapping tail
# (n0, r) per tile: 4 fill tiles, 23 mid tiles, 1 tail tile
TILES = (
    [(i * RS, RS) for i in range(NSMALL)]
    + [(NSMALL * RS + i * RF, RF) for i in range(NBIG - 1)]
    + [(NSHARD - RF, RF)]
)
NTILES = len(TILES)
SPLIT_NUM = 5  # DVE squares 5/16 of cols, ACT the rest

_CACHE = {}


def _build_bass():
    """Build + compile the per-core Bass module (one NEFF, SPMD on 8 cores)."""
    from contextlib import ExitStack

    import concourse.bacc as bacc
    import concourse.mybir as mybir
    import concourse.tile as tile
    from concourse.mybir import ActivationFunctionType as AFT

    f32 = mybir.dt.float32
    bf16 = mybir.dt.bfloat16
    f8 = mybir.dt.float8e4

    nc = bacc.Bacc("TRN2", target_bir_lowering=False, debug=False)

    xts = (
        nc.dram_tensor("xts", [NSMALL * P, C * RS], bf16, kind="ExternalInput").ap()
        if NSMALL
        else None
    )
    xtb = nc.dram_tensor("xtb", [NBIG * P, C * RF], bf16, kind="ExternalInput").ap()
    wb = nc.dram_tensor("wb", [P, C], bf16, kind="ExternalInput").ap()
    e8 = nc.dram_tensor("e8", [P, 32], f8, kind="ExternalInput").ap()
    oz = nc.dram_tensor("oz", [1, 2 * NS], bf16, kind="ExternalInput").ap()
    out = nc.dram_tensor("out", [NSHARD, NS], bf16, kind="ExternalOutput").ap()

    with tile.TileContext(nc) as tc, ExitStack() as ctx:
        const_pool = ctx.enter_context(tc.tile_pool(name="const", bufs=1))
        xt_pool = ctx.enter_context(tc.tile_pool(name="xt", bufs=4))
        sq_pool = ctx.enter_context(tc.tile_pool(name="sq", bufs=3))
        row_pool = ctx.enter_context(tc.tile_pool(name="rows", bufs=3))
        osb_pool = ctx.enter_context(tc.tile_pool(name="osb", bufs=3))
        pm_pool = ctx.enter_context(tc.tile_pool(name="pm", bufs=3, space="PSUM"))
        pv_pool = ctx.enter_context(tc.tile_pool(name="pv", bufs=3, space="PSUM"))
        po_pool = ctx.enter_context(tc.tile_pool(name="po", bufs=2, space="PSUM"))

        w_t = const_pool.tile([P, C], bf16)
        e8_t = const_pool.tile([P, 32], f8)
        oz_t = const_pool.tile([1, 2 * NS], bf16)

        xbt = {}
        sqt = {}
        meant = {}
        stdt = {}
        pot = {}

        def load(t):
            n0, r = TILES[t]
            xb_t = xt_pool.tile([P, C * r], bf16)
            if r == RS:
                nc.sync.dma_start(xb_t[:], xts[t * P : (t + 1) * P, :])
            elif t == 0:
                # quarter loads so tile 0's first mean matmul can start as
                # soon as 128KB has landed (pipeline fill ~5us faster);
                # downstream consumers are already per-quarter slices
                b = t - NSMALL
                for c in range(C):
                    nc.sync.dma_start(
                        xb_t[:, c * r : (c + 1) * r],
                        xtb[b * P : (b + 1) * P, c * r : (c + 1) * r],
                    )
            else:
                b = t - NSMALL
                nc.sync.dma_start(xb_t[:], xtb[b * P : (b + 1) * P, :])
            xbt[t] = xb_t

        def load_consts():
            nc.sync.dma_start(w_t[:], wb[:])
            nc.sync.dma_start(e8_t[:], e8[:])
            nc.sync.dma_start(oz_t[:], oz[:])

        def square(t):
            n0, r = TILES[t]
            xb_t = xbt[t]
            sq_t = sq_pool.tile([P, C * r], f8)
            if t == 0:
                # per-quarter, gated on the quarter loads
                for c in range(C):
                    sl = slice(c * r, (c + 1) * r)
                    if c % 2 == 0:
                        nc.vector.tensor_tensor(
                            sq_t[:, sl], xb_t[:, sl], xb_t[:, sl],
                            op=mybir.AluOpType.mult,
                        )
                    else:
                        nc.scalar.activation(sq_t[:, sl], xb_t[:, sl], AFT.Square)
            else:
                split = C * r * SPLIT_NUM // 16
                nc.vector.tensor_tensor(
                    sq_t[:, :split],
                    xb_t[:, :split],
                    xb_t[:, :split],
                    op=mybir.AluOpType.mult,
                )
                nc.scalar.activation(sq_t[:, split:], xb_t[:, split:], AFT.Square)
            sqt[t] = sq_t

        def reduce(t):
            n0, r = TILES[t]
            xb_t = xbt[t]
            sq_t = sqt[t]
            pm = pm_pool.tile([1, r], f32)
            for c in range(C):
                nc.tensor.matmul(
                    pm[:],
                    w_t[:, c : c + 1],
                    xb_t[:, c * r : (c + 1) * r],
                    start=(c == 0),
                    stop=(c == C - 1),
                )
            # var: DoubleRow fp8, chunk pair g = (2g, 2g+1) rides the Ko=2
            # AP dim: lhsT [128, 2, 1] (pair step 16B), rhs [128, 2, r]
            # (pair step = r bytes), virtual K=256 per matmul
            pv = pv_pool.tile([1, r], f32)
            e8_v = e8_t[:].rearrange("p (two g) -> p two g", two=2)
            for g in range(C // 2):
                nc.tensor.matmul(
                    pv[:],
                    e8_v[:, :, g : g + 1],
                    sq_t[:, 2 * g * r : 2 * (g + 1) * r].rearrange(
                        "p (two n) -> p two n", two=2
                    ),
                    start=(g == 0),
                    stop=(g == C // 2 - 1),
                    perf_mode=mybir.MatmulPerfMode.DoubleRow,
                )
            mean_t = row_pool.tile([1, r], bf16, tag="meanrow")
            nc.vector.tensor_copy(mean_t[:], pm[:])
            std_t = row_pool.tile([1, r], bf16, tag="stdrow")
            nc.scalar.sqrt(std_t[:], pv[:])
            meant[t] = mean_t
            stdt[t] = std_t

        def expand(t):
            n0, r = TILES[t]
            mean_t = meant.pop(t)
            std_t = stdt.pop(t)
            po = po_pool.tile([P, r], f32)
            if r == RF:
                # block q: rows n0 + 4p + q (quad-interleaved so the store
                # sees 1024B contiguous per partition in bf16)
                for q in range(4):
                    blk = po[:, q * P : (q + 1) * P]
                    lsl = slice(q, q + 4 * P - 3, 4)
                    nc.tensor.matmul(
                        blk, mean_t[:, lsl], oz_t[:, 0:NS], start=True, stop=False
                    )
                    nc.tensor.matmul(
                        blk, std_t[:, lsl], oz_t[:, NS : 2 * NS], start=False, stop=True
                    )
            else:
                nc.tensor.matmul(
                    po[:], mean_t[:], oz_t[:, 0:NS], start=True, stop=False
                )
                nc.tensor.matmul(
                    po[:], std_t[:], oz_t[:, NS : 2 * NS], start=False, stop=True
                )
            pot[t] = po

        def store(t):
            n0, r = TILES[t]
            po = pot.pop(t)
            osb = osb_pool.tile([P, r], bf16)
            nc.vector.tensor_copy(osb[:], po[:])
            if r == RF:
                # fused store; DRAM view strides: partition -> 4 rows,
                # inner (q j) -> 1024B contiguous
                nc.sync.dma_start(
                    out[n0 : n0 + r, :].rearrange("(p q) j -> p (q j)", p=P, q=4),
                    osb[:],
                )
            else:
                nc.sync.dma_start(out[n0 : n0 + r, :], osb[:])
            xbt.pop(t)
            sqt.pop(t)

        # software pipeline: load k | square k-1 | reduce k-2 | out k-3
        for k in range(NTILES + 3):
            if k == 0:
                load(0)
                load(1)
                load_consts()
            elif 2 <= k < NTILES:
                load(k)
            if 1 <= k < NTILES + 1:
                square(k - 1)
            if 2 <= k < NTILES + 2:
                reduce(k - 2)
            if 3 <= k < NTILES + 3:
                expand(k - 3)
                store(k - 3)

    nc.compile()
    return nc


def _host_inputs(x, w_mu, w_log_var, z):
    import ml_dtypes

    bf16 = ml_dtypes.bfloat16
    f8 = ml_dtypes.float8_e4m3fn

    xb = x.astype(bf16)  # [N, D]
    wb = np.ascontiguousarray(w_mu.astype(np.float32).reshape(C, P).T).astype(bf16)
    e = np.exp(w_log_var.astype(np.float32))
    e8 = np.zeros((P, 32), dtype=f8)
    for g in range(C // 2):
        for two in range(2):
            c = 2 * g + two
            e8[:, two * 16 + g] = e[c * P : (c + 1) * P].astype(f8)
    oz = np.empty((1, 2 * NS), dtype=bf16)
    oz[0, :NS] = 1.0
    oz[0, NS:] = z.astype(bf16)

    def slab(rows):  # [r, D] -> [P, C*r]
        r = rows.shape[0]
        return rows.reshape(r, C, P).transpose(2, 1, 0).reshape(P, C * r)

    ins = []
    for cid in range(NCORES):
        xs = xb[cid * NSHARD : (cid + 1) * NSHARD]
        small = np.empty((NSMALL, P, C * RS), dtype=bf16)
        big = np.empty((NBIG, P, C * RF), dtype=bf16)
        for i, (n0, r) in enumerate(TILES):
            if r == RS:
                small[i] = slab(xs[n0 : n0 + r])
            else:
                big[i - NSMALL] = slab(xs[n0 : n0 + r])
        m = {
            "xtb": big.reshape(NBIG * P, C * RF),
            "wb": wb,
            "e8": e8,
            "oz": oz,
        }
        if NSMALL:
            m["xts"] = small.reshape(NSMALL * P, C * RS)
        ins.append(m)
    return ins


def _get_nc():
    if "nc" not in _CACHE:
        _CACHE["nc"] = _build_bass()
    return _CACHE["nc"]


def kernel(x, w_mu, w_log_var, z, _trace=False, _tmpdir=None):
    from concourse.bass_utils import run_bass_kernel_spmd

    x = np.ascontiguousarray(x, dtype=np.float32)
    w_mu = np.asarray(w_mu, dtype=np.float32)
    w_log_var = np.asarray(w_log_var, dtype=np.float32)
    z = np.asarray(z, dtype=np.float32)

    in_maps = _host_inputs(x, w_mu, w_log_var, z)

    nc = _get_nc()
    res = run_bass_kernel_spmd(
        nc,
        in_maps,
        core_ids=list(range(NCORES)),
        trace=_trace,
        tmpdir=_tmpdir,
        stitch_traces=False,
    )
    _CACHE["last_results"] = res
    outs = [r["out"] for r in res.results]
    return np.concatenate(outs, axis=0).astype(np.float32)
